# revision 63
# baseline (speedup 1.0000x reference)
"""Multi-head attention (B=4,S=2048,D=1024,H=16,dh=65) on 8 TRN2 NeuronCores.

Sharding: batch x head-half. Core c handles batch c//2 and heads
(c%2)*8..(c%2)*8+8 (P-slice of 520). Each core computes its QKV projections,
attention, and a partial out-projection; the host sums the two partials per
batch and adds bo.

v4 structure:
 - Q/K projections computed directly in transposed [dh, S] layout (M=65
   matmuls, bias folded into the PSUM->SBUF drain via DVE tensor_scalar_add);
   no PE transposes anywhere, keeping the HAM activity monitor seeing dense
   matmul traffic (a transpose-heavy phase runs at K=4/8 half clock).
 - Attention inner loop is software-pipelined in COMMIT order (the tensor
   engine queue is strict FIFO): scores(r) -> interleave closures (q-proj /
   norm / out-proj work spread across rounds) -> AV(r-1) -> exp(r) -> mask(r).
   This keeps ready matmuls ahead of the AV that waits on the exp chain.
 - The per-qb epilogue (reciprocal -> rbp broadcast -> cch -> concat pack ->
   out-proj) is scheduled to avoid head-of-line blocking: slot 0 commits only
   the DVE reciprocal; norms run in slots 1-4, out-proj in slots 5-7 plus one
   spilled to the next qb; the final qb runs the chain split into q-halves.
 - rbp broadcast matmul in bf16 (fp32 matmuls double-pump LOW/HIGH at ~4x
   cost).

Compute dtype bf16 (fp32 PSUM accumulation); softmax runs unnormalized
(no max subtraction -- score magnitudes are bounded ~20 so exp stays in fp32
range) with the row-sum harvested from a trailing ones-column in V.
"""

import math
import sys
from contextlib import ExitStack

import numpy as np
import ml_dtypes

sys.path.insert(0, "/opt/trn_rl_repo")

import concourse.bass as bass
import concourse.mybir as mybir
import concourse.tile as tile_mod
from concourse.bass_utils import run_bass_kernel_spmd
from concourse.vector_clock import ScopedClock

# ---------------------------------------------------------------------------
# Patch for this container's walrus build: it rejects instructions carrying
# more than one semaphore wait ("Too many sync wait commands"), but Tile's
# wait assigner freely attaches several. Split excess waits onto bass_nofuse
# InstNoOp carriers on the same engine, committed immediately before the
# instruction (same-engine program order => over-synchronization only).
# ---------------------------------------------------------------------------
_MAX_WAITS = 1

_orig_commit = tile_mod.TileContext._commit_instruction


def _split_waits(self, inst, commit):
    si = inst.sync_info
    if si is None or len(si.on_wait) <= _MAX_WAITS:
        return
    waits = list(si.on_wait)
    sem_w = [w for w in waits if getattr(w, "sync_type", "semaphore") == "semaphore"]
    other_w = [w for w in waits if getattr(w, "sync_type", "semaphore") != "semaphore"]
    keep_budget = _MAX_WAITS - len(other_w)
    if keep_budget < 0:
        return
    keep = other_w + (sem_w[-keep_budget:] if keep_budget > 0 else [])
    excess = sem_w[: len(sem_w) - max(keep_budget, 0)]
    if not excess:
        return
    for i, w in enumerate(excess):
        nop = mybir.InstNoOp(
            name=f"{inst.name}-sw{i}",
            sync_info=mybir.SyncInfo(on_wait=[w], on_update=[]),
            bass_nofuse=True,
            engine=inst.engine,
        )
        commit(nop)
    inst.sync_info = mybir.SyncInfo(on_wait=keep, on_update=list(si.on_update))


def _patched_commit(self, inst, lazy_reg_writes: bool = True):
    if inst.engine != mybir.EngineType.Unassigned:
        _split_waits(self, inst, lambda n: _orig_commit(self, n, False))
    return _orig_commit(self, inst, lazy_reg_writes)


def _patched_drain_and_barrier(self, tick_clock, wait_clock):
    drain_inst = self.nc.sync.drain()
    wait_clock.add_sem_waits(
        drain_inst.ins, ScopedClock({None: tick_clock.global_clock})
    )
    si = drain_inst.ins.sync_info
    if si is not None and len(si.on_wait) > _MAX_WAITS:
        waits = list(si.on_wait)
        drain_inst.ins.sync_info = mybir.SyncInfo(
            on_wait=waits[:_MAX_WAITS], on_update=list(si.on_update)
        )
        for w in waits[_MAX_WAITS:]:
            n = self.nc.sync.nop(nofuse=True)
            n.ins.sync_info = mybir.SyncInfo(on_wait=[w], on_update=[])
    self.nc.all_engine_barrier()
    popped = self.nc._tile_sem_poison_stack.pop()
    assert popped is self._sem_poison
    self.nc.clear_and_free_semaphores(list(self.sems.allocated().values()))
    self.nc.all_engine_barrier()


tile_mod.TileContext._commit_instruction = _patched_commit
tile_mod.TileContext._drain_and_barrier = _patched_drain_and_barrier

# ---------------------------------------------------------------------------

B, S, D, H = 4, 2048, 1024, 16
DH = D // H + 1          # 65
P = H * DH               # 1040
HPC = H // 2             # heads per core
PC = HPC * DH            # 520, per-core P slice
N_CORES = 8

MT = S // 128            # 16 row blocks / k tiles
KT = 16                  # k tiles per attention
QB = 4                   # q blocks of 512
QW = 512
RKT = 2                  # k-tiles per score round (2 banks, double-buffered)
NR = KT // RKT           # 8 rounds

F32 = mybir.dt.float32
BF16 = mybir.dt.bfloat16
BF = ml_dtypes.bfloat16

_BUILT = {}


def _build_nc():
    nc = bass.Bass("TRN2", target_bir_lowering=False, debug=False,
                   num_devices=N_CORES)

    xq_d = nc.dram_tensor("xq", [D, S], BF16, kind="ExternalInput").ap()
    xk_d = nc.dram_tensor("xk", [D, S], BF16, kind="ExternalInput").ap()
    xv_d = nc.dram_tensor("xv", [D, S], BF16, kind="ExternalInput").ap()
    # maskH[qb, p, j*QW+q] = maskT[j*128+p, qb*512+q] (multiplicative 0/1)
    mh = nc.dram_tensor("maskH", [QB, 128, KT * QW], BF16,
                        kind="ExternalInput").ap()
    wq_d = nc.dram_tensor("wqT", [D, PC], BF16, kind="ExternalInput").ap()
    wk_d = nc.dram_tensor("wkT", [D, PC], BF16, kind="ExternalInput").ap()
    wv_d = nc.dram_tensor("wvT", [D, PC], BF16, kind="ExternalInput").ap()
    # per-head bias columns: [dh, head]
    bqT_d = nc.dram_tensor("bqT", [DH, HPC], F32, kind="ExternalInput").ap()
    bkT_d = nc.dram_tensor("bkT", [DH, HPC], F32, kind="ExternalInput").ap()
    bv_d = nc.dram_tensor("bv", [1, PC], BF16, kind="ExternalInput").ap()
    wo_d = nc.dram_tensor("woT", [PC, D], BF16, kind="ExternalInput").ap()
    sel8_d = nc.dram_tensor("sel8", [HPC, HPC * DH], BF16,
                            kind="ExternalInput").ap()
    out = nc.dram_tensor("out", [S, D], F32, kind="ExternalOutput").ap()

    # packed 128-row k-tile ranges of the 520-row concatT / WoT
    PKT = [(0, 128), (128, 256), (256, 384), (384, 512), (512, 520)]
    inv_sqrt = 1.0 / math.sqrt(float(DH))

    with tile_mod.TileContext(nc) as tc:
        with tc.tile_pool(name="const", bufs=1) as pconst, \
             tc.tile_pool(name="qkT", bufs=1) as pqkT, \
             tc.tile_pool(name="vh", bufs=MT + 1) as pvh, \
             tc.tile_pool(name="psS", bufs=2, space="PSUM") as psS, \
             tc.tile_pool(name="psA", bufs=4, space="PSUM") as psA:

            ones_col = pconst.tile([1, 128], BF16, tag="ones")
            nc.gpsimd.memset(ones_col[:], 1.0)
            # warm up the ACT exp table set (first EXP otherwise pays the
            # ~2.7us ACT_TABLE_LOAD mid-attention)
            scratch = pconst.tile([1, 128], BF16, tag="scratch")
            nc.scalar.activation(scratch[:], ones_col[:],
                                 mybir.ActivationFunctionType.Exp)
            sel8 = pconst.tile([HPC, HPC * DH], BF16, tag="sel8")
            bqT = pconst.tile([DH, HPC], F32, tag="bqT")
            bkT = pconst.tile([DH, HPC], F32, tag="bkT")

            # K transposed: [65, head, S]; Q transposed lives in a 2-deep
            # ring of per-qb tiles [65, head, 512]
            kT = pqkT.tile([DH, HPC, S], BF16, tag="kT")
            # v k-tiles with trailing ones column: [128, head, 65+1]
            vh = [pvh.tile([128, HPC, DH + 1], BF16, tag="vh", name=f"vh{j}")
                  for j in range(MT)]
            for j in range(MT):
                nc.gpsimd.memset(vh[j][:, :, DH:DH + 1], 1.0)

            _stack = ExitStack()
            pw = _stack.enter_context(tc.tile_pool(name="pw", bufs=9))
            pb = _stack.enter_context(tc.tile_pool(name="pb", bufs=1))
            pxc = _stack.enter_context(tc.tile_pool(name="pxc", bufs=16))
            pqT = _stack.enter_context(tc.tile_pool(name="pqT", bufs=2))
            pm = _stack.enter_context(tc.tile_pool(name="pm", bufs=3))
            _xkv_stack = ExitStack()
            pxkv = _xkv_stack.enter_context(tc.tile_pool(name="pxkv", bufs=17))
            # head-0 slice of Wq, loaded up front so the first q-projection
            # does not wait for the full Wq DMA (whose ring slots serialize
            # behind the K-projection readers)
            wq0_t = [pb.tile([128, 2 * DH], BF16, tag=f"wq0_{d}",
                             name=f"wq0_{d}") for d in range(8)]

            # ---- phase A: V projection (row layout, full-util matmuls) ----
            wv_t = []
            for d in range(8):
                wt = pw.tile([128, HPC, DH], BF16, tag="w", name=f"wv{d}")
                nc.sync.dma_start(wt[:], wv_d[d * 128:(d + 1) * 128, :])
                wv_t.append(wt)
            bv_t = pb.tile([1, HPC, DH], BF16, tag="b", name="bv_t")
            nc.sync.dma_start(bv_t[:], bv_d[:])
            xv_t = []
            for d in range(8):
                xt = pxkv.tile([128, S], BF16, tag="xkv", name=f"xv{d}")
                xv_t.append(xt)
            for qu in range(4):
                for d in range(8):
                    nc.sync.dma_start(
                        xv_t[d][:, qu * 512:(qu + 1) * 512],
                        xv_d[d * 128:(d + 1) * 128, qu * 512:(qu + 1) * 512])
            # small constants behind the V-projection's gating loads
            for d in range(8):
                nc.sync.dma_start(wq0_t[d][:],
                                  wq_d[d * 128:(d + 1) * 128, 0:2 * DH])
            nc.sync.dma_start(sel8[:], sel8_d[:])
            nc.sync.dma_start(bqT[:], bqT_d[:])
            nc.sync.dma_start(bkT[:], bkT_d[:])

            def vproj_m(m):
                for half in range(2):
                    hs = half * 4
                    ps = psA.tile([128, 4, DH], F32, tag="psA",
                                  name=f"pv{m}_{half}")
                    nc.tensor.matmul(ps[:], ones_col[0:1, :],
                                     bv_t[0:1, hs:hs + 4, :],
                                     start=True, stop=False)
                    for d in range(8):
                        c0 = m * 128
                        nc.tensor.matmul(ps[:], xv_t[d][:, c0:c0 + 128],
                                         wv_t[d][:, hs:hs + 4, :],
                                         start=False, stop=(d == 7))
                    nc.vector.tensor_copy(vh[m][:, hs:hs + 4, 0:DH], ps[:])

            # ---- phase B helpers: direct-transposed Q/K projection --------
            def tproj_head(dst, h, sb, x_t, w_t, bT, tag=""):
                """dst = (W x)^T + b for head h, s-block sb; M=65 matmuls."""
                ps = psA.tile([DH, QW], F32, tag="psA", name=f"tp{tag}_{h}_{sb}")
                for d in range(8):
                    nc.tensor.matmul(
                        ps[:], w_t[d][:, h * DH:(h + 1) * DH],
                        x_t[d][:, sb * QW:(sb + 1) * QW],
                        start=(d == 0), stop=(d == 7))
                nc.vector.tensor_scalar_add(dst, ps[:], bT[:, h:h + 1])

            # issue K-side loads up front (pxkv has room for xv+xk)
            wk_t = []
            for d in range(8):
                wt = pw.tile([128, PC], BF16, tag="wf", name=f"wk{d}")
                nc.sync.dma_start(wt[:], wk_d[d * 128:(d + 1) * 128, :])
                wk_t.append(wt)
            xk_t = []
            for d in range(8):
                xt = pxkv.tile([128, S], BF16, tag="xkv", name=f"xk{d}")
                xk_t.append(xt)
            for half in range(2):
                for d in range(8):
                    nc.sync.dma_start(
                        xk_t[d][:, half * 1024:(half + 1) * 1024],
                        xk_d[d * 128:(d + 1) * 128,
                             half * 1024:(half + 1) * 1024])

            def load_xq_chunk(qc):
                xts = []
                for d in range(8):
                    xt = pxc.tile([128, QW], BF16, tag="xqc",
                                  name=f"xqc{qc}_{d}")
                    nc.sync.dma_start(
                        xt[:], xq_d[d * 128:(d + 1) * 128,
                                    qc * QW:(qc + 1) * QW])
                    xts.append(xt)
                return xts

            xq_c = load_xq_chunk(0)

            def load_mask(qb):
                mts = []
                for hf in range(2):
                    mt = pm.tile([128, KT // 2, QW], BF16, tag="mask",
                                 name=f"mask{qb}_{hf}")
                    nc.sync.dma_start(
                        mt[:], mh[qb, :, hf * (KT // 2) * QW:
                                  (hf + 1) * (KT // 2) * QW])
                    mts.append(mt)
                return mts

            mts0 = load_mask(0)

            for m in range(MT):
                vproj_m(m)
            for sb in range(4):
                for h in range(HPC):
                    tproj_head(kT[0:DH, h, sb * QW:(sb + 1) * QW],
                               h, sb, xk_t, wk_t, bkT, tag="k")
            # wq reuses the wk ring slots -- issue after K proj is committed
            # so Tile sees the wk readers when sequencing the overwrites.
            wq_t = []
            for d in range(8):
                wt = pw.tile([128, PC], BF16, tag="wf", name=f"wq{d}")
                nc.sync.dma_start(wt[:], wq_d[d * 128:(d + 1) * 128, :])
                wq_t.append(wt)
            # q0 projection of head 0 only (from the early Wq slice); heads
            # 1-7 are projected inside qb0's attention slots, hiding the
            # full-Wq DMA behind the first attention rounds
            qT0 = pqT.tile([DH, HPC, QW], BF16, tag="qT", name="qT0")
            tproj_head(qT0[0:DH, 0, :], 0, 0, xq_c, wq0_t, bqT, tag="q0")
            _xkv_stack.close()  # free the 64KB x-tile pool before phase 2

            # ---------------- phase 2+3 ------------------------------------
            with tc.tile_pool(name="pp", bufs=3) as pp, \
                 tc.tile_pool(name="pc", bufs=1) as pc, \
                 tc.tile_pool(name="pwo", bufs=1) as pwo, \
                 tc.tile_pool(name="po", bufs=2) as po, \
                 tc.tile_pool(name="pt2", bufs=4) as pt2:

                # packed concatT: 128-row tiles covering rows 0..520
                ccp = [pc.tile([b - a, S], BF16, tag=f"ccp{i}",
                               name=f"ccp{i}")
                       for i, (a, b) in enumerate(PKT)]
                wop = []
                for i, (a, b) in enumerate(PKT):
                    w = pwo.tile([b - a, D], BF16, tag=f"wop{i}",
                                 name=f"wop{i}")
                    nc.sync.dma_start(w[:], wo_d[a:b, :])
                    wop.append(w)

                state = {}

                # ---- interleavable work units (committed between attention
                # rounds so the tensor FIFO never head-of-line blocks) ------
                def qproj_closures(h, xq_t, qT_next, tag, w_t=None):
                    box = {}
                    def mk(i):
                        def f():
                            wt = w_t if w_t is not None else wq_t
                            if i == 0:
                                box["ps"] = psA.tile(
                                    [DH, QW], F32, tag="psA",
                                    name=f"qp{tag}_{h}")
                            ps = box["ps"]
                            for d in (2 * i, 2 * i + 1):
                                nc.tensor.matmul(
                                    ps[:], wt[d][:, h * DH:(h + 1) * DH],
                                    xq_t[d][:], start=(d == 0), stop=(d == 7))
                            if i == 3:
                                nc.vector.tensor_scalar_add(
                                    qT_next[0:DH, h, :], ps[:],
                                    bqT[:, h:h + 1])
                        return f
                    return [mk(i) for i in range(4)]

                def outproj_closures(m, scalar_drain=False):
                    box = {}
                    def mk(n, piece):
                        def f():
                            if piece == 0:
                                box[n] = psA.tile([128, QW], F32, tag="psA",
                                                  name=f"psop{m}_{n}")
                                rng = range(0, 3)
                            else:
                                rng = range(3, 5)
                            ps = box[n]
                            for i in rng:
                                nc.tensor.matmul(
                                    ps[:], ccp[i][:, m * 128:(m + 1) * 128],
                                    wop[i][:, n * QW:(n + 1) * QW],
                                    start=(i == 0), stop=(i == 4))
                            if piece == 1:
                                osb = po.tile([128, QW], F32, tag="osb",
                                              name=f"osb{m}_{n}", bufs=3)
                                if scalar_drain and n == 0:
                                    nc.scalar.copy(osb[:], ps[:])
                                else:
                                    nc.vector.tensor_copy(osb[:], ps[:])
                                # tail DMAs all on sync: the gpsimd SWDGE
                                # path is ~3x slower and sits at the very
                                # end of the kernel
                                eng = nc.sync if scalar_drain else (
                                    nc.gpsimd if n else nc.sync)
                                eng.dma_start(
                                    out[m * 128:(m + 1) * 128,
                                        n * QW:(n + 1) * QW],
                                    osb[:])
                        return f
                    return [mk(0, 0), mk(0, 1), mk(1, 0), mk(1, 1)]

                HQ = QW // 2

                def recip_closure(qb):
                    # column-split halves: the first rbp broadcast only
                    # needs the first 1.74us half-reciprocal instead of the
                    # full 3.34us one (free-dim-bound DVE op)
                    st = state[qb]
                    def f():
                        rcs = []
                        for half in range(2):
                            c0 = half * HQ
                            rc32 = pt2.tile([HPC, HQ], F32, tag="rc32",
                                            name=f"rc32_{qb}_{half}", bufs=2)
                            nc.vector.reciprocal(rc32[:],
                                                 st["rsall"][:, c0:c0 + HQ])
                            rch = pt2.tile([HPC, HQ], BF16, tag="rcall",
                                           name=f"rcall{qb}_{half}", bufs=4)
                            nc.vector.tensor_copy(rch[:], rc32[:])
                            rcs.append(rch)
                        st["rcall"] = rcs
                    return f

                def norm_head_do(qb, h, rbp, cch, rc, c0, cw):
                    """cch cols [c0,c0+cw) = uov * (1/rowsum), packed into
                    the concatT tiles via row-shifting DMAs (full width)."""
                    nc.tensor.matmul(rbp[0:DH, c0:c0 + cw],
                                     sel8[:, h * DH:(h + 1) * DH], rc,
                                     start=True, stop=True)
                    nc.vector.tensor_mul(cch[0:DH, c0:c0 + cw],
                                         rbp[0:DH, c0:c0 + cw],
                                         state[qb]["uovs"][h][:, c0:c0 + cw])

                def pack_cch(qb, h, cch, c0, cw):
                    r0 = h * DH
                    for i, (a, b) in enumerate(PKT):
                        lo, hi = max(r0, a), min(r0 + DH, b)
                        if lo < hi:
                            # sync queue only: gpsimd is reserved for the
                            # latency-critical rowsum gather DMAs
                            nc.sync.dma_start(
                                ccp[i][lo - a:hi - a,
                                       qb * QW + c0:qb * QW + c0 + cw],
                                cch[lo - r0:hi - r0, c0:c0 + cw])

                def norm_head_closure(qb, h):
                    def f():
                        # both halves share one PSUM bank: two independent
                        # single matmuls never accumulate, so the whole-bank
                        # has_written clear of the second start=True is
                        # harmless to the first half's values
                        rbp = psA.tile([128, QW], F32, tag="psA",
                                       name=f"rbp{qb}_{h}")
                        cch = pt2.tile([DH, QW], BF16, tag="cch",
                                       name=f"cch{qb}_{h}", bufs=3)
                        for half in range(2):
                            norm_head_do(qb, h, rbp, cch,
                                         state[qb]["rcall"][half][:],
                                         half * HQ, HQ)
                        pack_cch(qb, h, cch, 0, QW)
                    return f

                # ---- attention head slot (software-pipelined commits) -----
                def attn_head(qb, h, qTc, mts, rsall, uovs, inter):
                    ov = psA.tile([128, QW], F32, tag="psA",
                                  name=f"ov{qb}_{h}")

                    def av_round(pt, r):
                        for jj in range(RKT):
                            j = r * RKT + jj
                            nc.tensor.matmul(
                                ov[0:DH + 1, :], vh[j][:, h, :],
                                pt[:, jj, :],
                                start=(j == 0), stop=(j == KT - 1))

                    ii = 0
                    pend = None
                    for r in range(NR):
                        ss = psS.tile([128, RKT, QW], F32, tag="psS",
                                      name=f"ss{qb}_{h}_{r}")
                        for jj in range(RKT):
                            j = r * RKT + jj
                            nc.tensor.matmul(
                                ss[:, jj, :],
                                kT[0:DH, h, j * 128:(j + 1) * 128],
                                qTc[0:DH, h, :],
                                start=True, stop=True)
                        take = -((ii - len(inter)) // (NR - r))  # ceil
                        for _ in range(take):
                            if inter[ii] is not None:
                                inter[ii]()
                            ii += 1
                        if pend is not None:
                            av_round(*pend)
                        pt = pp.tile([128, RKT, QW], BF16, tag="pT",
                                     name=f"pt{qb}_{h}_{r}")
                        nc.scalar.activation(
                            pt[:], ss[:],
                            mybir.ActivationFunctionType.Exp,
                            scale=inv_sqrt)
                        mt = mts[r // (NR // 2)]
                        rr = r % (NR // 2)
                        nc.vector.tensor_mul(
                            pt[:], pt[:], mt[:, rr * RKT:(rr + 1) * RKT, :])
                        pend = (pt, r)
                    av_round(*pend)
                    uov = pt2.tile([DH, QW], BF16, tag="uov",
                                   name=f"uov{qb}_{h}", bufs=2 * HPC + 1)
                    nc.vector.tensor_copy(uov[:], ov[0:DH, :])
                    rs2 = pt2.tile([66, QW], F32, tag="rs2",
                                   name=f"rs2_{qb}_{h}", bufs=2)
                    if h == HPC - 1:
                        # last head's rowsum gates the reciprocal chain --
                        # use the scalar engine to skip the DVE queue backlog
                        nc.scalar.copy(rs2[64:66, :], ov[64:66, :])
                    else:
                        nc.vector.tensor_copy(rs2[64:66, :], ov[64:66, :])
                    nc.gpsimd.dma_start(rsall[h:h + 1, :], rs2[65:66, :])
                    uovs.append(uov)

                next_mts = mts0
                qT_cur = qT0
                for qb in range(QB):
                    mts = next_mts
                    rsall = pt2.tile([HPC, QW], F32, tag="rsall",
                                     name=f"rsall{qb}", bufs=2)
                    uovs = []
                    state[qb] = {"rsall": rsall, "uovs": uovs}
                    for h in range(HPC):
                        inter = []
                        # q-projections run one slot ahead of their use;
                        # head h+1 of qb0 in slot h (head 1 from the early
                        # Wq slice so nothing waits on the full Wq DMA),
                        # head h-1 of qb+1 in slot h>=1, head 7 spilling
                        # into the next qb's slot 0.
                        if qb == 0:
                            if h == 0:
                                inter += qproj_closures(1, xq_c, qT0,
                                                        tag="q0", w_t=wq0_t)
                            elif h <= HPC - 2:
                                inter += qproj_closures(h + 1, xq_c, qT0,
                                                        tag="q0")
                        if qb >= 1 and h == 0:
                            inter += qproj_closures(HPC - 1, state["xq"],
                                                    qT_cur, tag=f"q{qb}s")
                        if qb < QB - 1:
                            if h == 0:
                                state["xq"] = load_xq_chunk(qb + 1)
                                state["qT_next"] = pqT.tile(
                                    [DH, HPC, QW], BF16, tag="qT",
                                    name=f"qT{qb + 1}")
                            else:
                                inter += qproj_closures(
                                    h - 1, state["xq"], state["qT_next"],
                                    tag=f"q{qb + 1}")
                        if qb > 0:
                            # norms deferred to slots 3-6 so the rowsum ->
                            # reciprocal -> rcall chain (committed in slot 0)
                            # has ~3 slots of slack before the first rbp
                            # matmul reaches the tensor FIFO head
                            if h == 0:
                                inter.append(recip_closure(qb - 1))
                                if qb >= 2:
                                    inter += outproj_closures((qb - 2) * 4 + 1)
                            elif h <= 2:
                                if qb >= 2:
                                    inter += outproj_closures((qb - 2) * 4 + 1 + h)
                            elif h <= 6:
                                # pad so the rbp matmuls land in rounds 6-7,
                                # well after the reciprocal chain resolves
                                inter += [None] * max(0, 6 - len(inter))
                                inter += [norm_head_closure(qb - 1, 2 * (h - 3)),
                                          norm_head_closure(qb - 1, 2 * h - 5)]
                            else:
                                inter += outproj_closures((qb - 1) * 4)
                        attn_head(qb, h, qT_cur, mts, rsall, uovs, inter)
                        if qb < QB - 1 and h == 1:
                            next_mts = load_mask(qb + 1)
                    if qb < QB - 1:
                        qT_cur = state["qT_next"]

                # ---- tail: spilled out-proj (pure tensor work that fills
                # the pipe while the last qb's reciprocal chain resolves),
                # then the last qb's norm and out-proj in q-halves ----------
                rchs = []
                for half in range(2):
                    c0 = half * HQ
                    rc32 = pt2.tile([HPC, HQ], F32, tag="rc32h",
                                    name=f"rc32t{half}", bufs=2)
                    nc.vector.reciprocal(rc32[:],
                                         state[QB - 1]["rsall"][:, c0:c0 + HQ])
                    rch = pt2.tile([HPC, HQ], BF16, tag="rcallh",
                                   name=f"rcallt{half}", bufs=2)
                    nc.vector.tensor_copy(rch[:], rc32[:])
                    rchs.append(rch)
                for m in range((QB - 2) * 4 + 1, (QB - 1) * 4):
                    for f in outproj_closures(m, scalar_drain=True):
                        f()
                for half in range(2):
                    c0 = half * HQ
                    for h in range(HPC):
                        rbp = psA.tile([128, QW], F32, tag="psA",
                                       name=f"rbpt{h}_{half}")
                        cch = pt2.tile([DH, QW], BF16, tag="cch",
                                       name=f"ccht{h}_{half}", bufs=3)
                        norm_head_do(QB - 1, h, rbp, cch, rchs[half][:],
                                     c0, HQ)
                        pack_cch(QB - 1, h, cch, c0, HQ)
                    for mm_ in range(2):
                        m = (QB - 1) * 4 + 2 * half + mm_
                        for f in outproj_closures(m, scalar_drain=True):
                            f()
            _stack.close()

    return nc


def _prep_inputs(q, k, v, mask, Wq, bqv, Wk, bkv, Wv, bvv, Wo):
    """Per-core input maps (numpy, host-side shard + cast)."""
    in_maps = []
    sel8 = np.zeros((HPC, HPC * DH), np.float32)
    for h in range(HPC):
        sel8[h, h * DH:(h + 1) * DH] = 1.0
    mask_h = {}
    for b in range(B):
        mt = (mask[b, 0] != 0).astype(np.float32).T  # [k, q]
        m4 = mt.reshape(KT, 128, QB, QW).transpose(2, 1, 0, 3)
        mask_h[b] = np.ascontiguousarray(m4.reshape(QB, 128, KT * QW)).astype(BF)
    for c in range(N_CORES):
        b, hh = c // 2, c % 2
        sl = slice(hh * PC, (hh + 1) * PC)
        in_maps.append({
            "xq": np.ascontiguousarray(q[b].T).astype(BF),
            "xk": np.ascontiguousarray(k[b].T).astype(BF),
            "xv": np.ascontiguousarray(v[b].T).astype(BF),
            "maskH": mask_h[b],
            "wqT": np.ascontiguousarray(Wq[sl, :].T).astype(BF),
            "wkT": np.ascontiguousarray(Wk[sl, :].T).astype(BF),
            "wvT": np.ascontiguousarray(Wv[sl, :].T).astype(BF),
            "bqT": np.ascontiguousarray(
                bqv[sl].reshape(HPC, DH).T).astype(np.float32),
            "bkT": np.ascontiguousarray(
                bkv[sl].reshape(HPC, DH).T).astype(np.float32),
            "bv": bvv[sl].reshape(1, PC).astype(BF),
            "woT": np.ascontiguousarray(Wo[:, sl].T).astype(BF),
            "sel8": sel8.astype(BF),
        })
    return in_maps


def run_sharded(in_maps, **kwargs):
    if "nc" not in _BUILT:
        _BUILT["nc"] = _build_nc()
    return run_bass_kernel_spmd(_BUILT["nc"], in_maps,
                                core_ids=list(range(N_CORES)), **kwargs)


def kernel(q, k, v, mask, Wq, bq, Wk, bk, Wv, bv, Wo, bo):
    q = np.asarray(q, np.float32)
    k = np.asarray(k, np.float32)
    v = np.asarray(v, np.float32)
    mask = np.asarray(mask)
    in_maps = _prep_inputs(q, k, v, mask,
                           np.asarray(Wq, np.float32), np.asarray(bq, np.float32),
                           np.asarray(Wk, np.float32), np.asarray(bk, np.float32),
                           np.asarray(Wv, np.float32), np.asarray(bv, np.float32),
                           np.asarray(Wo, np.float32))
    res = run_sharded(in_maps)
    bo32 = np.asarray(bo, np.float32)
    out = np.empty((B, S, D), np.float32)
    for b in range(B):
        out[b] = res.results[2 * b]["out"] + res.results[2 * b + 1]["out"] + bo32
    return out
